# revision 33
# baseline (speedup 1.0000x reference)
"""Tensor-parallel fused attention kernel for Trainium2 (8 NeuronCores).

Sharding: DP=2 over batch x TP=4 over kv-head pairs. Each core computes
q/k/v projections + RoPE + causal attention + output projection for its
(batch, 2 kv heads) shard in bf16, then a 4-core ReduceScatter combines
the partial output projections; the host assembles the disjoint row
shards into the full [2, 2048, 4096] output.

Causal fast path computes attention scores transposed (S^T = K^T Q per
block, partition dim = key position): y = expS^T . [V|1] with a ones
column appended to V so the softmax denominator falls out of the same
matmul. This removes the per-block P transposes and the exp-sum/
reciprocal/diag chain from the PE critical path; only one small
[i,H]->[H,i] transpose per output tile remains before the out-proj.
"""
import sys

for _p in ("/opt/trn_rl_repo", "/root/.axon_site/_ro/trn_rl_repo"):
    if _p not in sys.path:
        sys.path.append(_p)

import math
import numpy as np
import ml_dtypes

import concourse.bass as bass
import concourse.mybir as mybir
import concourse.tile as tile
from concourse import bacc
from concourse import bass_utils
from concourse.masks import make_identity

BF16 = ml_dtypes.bfloat16
FP32 = mybir.dt.float32
BF = mybir.dt.bfloat16

B, S, D = 2, 2048, 4096
R, K, H = 4, 8, 128
N_CORES = 8
TP = 4            # tensor-parallel ways (kv-head axis)
KLOC = K // TP    # kv heads per core = 2
HEADS = R * KLOC  # query heads per core = 8
DT = D // 128     # 32 d-tiles
ST = S // 128     # 16 s-tiles
HP = H + 1        # v columns per kv head incl. ones column
NQ = 4            # x streamed in 4 quarters of 512 sequence cols

# 4-tile attention segments; each segment's out-projection is emitted
# interleaved into the NEXT segment's head loop so exp (ACT) overlaps the
# out-proj matmuls and the ReduceScatter chunks fire a segment early.
# Order: (0,4) first (cheap attention, PE-bound start), then (12,4) whose
# heavy exp load hides under seg(0,4)'s interleaved out-proj; last-processed
# rows 8-11 use single-tile chunks, the final one column-split, to minimize
# the serial RS tail.
SEGMENTS = [(0, 4), (12, 4), (4, 4), (8, 4)]
# ReduceScatter chunks (start_tile, n_tiles); fired as soon as the
# out-projection rows for the chunk are complete. Two-tile chunks
# minimize total CC time (the end chain is CC-throughput-bound).
CC_CHUNKS = [(0, 2), (2, 2), (12, 2), (14, 2), (4, 2), (6, 2), (8, 2),
             (10, 2)]
SPLIT_TILE = -1  # column-split of the final chunk disabled

_CACHE = {}


def _build_fast():
    """Causal-mask fast path."""
    nc = bacc.Bacc("TRN2", target_bir_lowering=False, debug=False,
                   enable_asserts=False, num_devices=N_CORES)

    # x quarter-major: col = q*(DT*512) + d*512 + s_local
    xP = nc.dram_tensor("xP", [128, NQ * DT * 512], BF, kind="ExternalInput")
    wq = nc.dram_tensor("wq", [HEADS * 128, DT * 128], BF, kind="ExternalInput")
    wk = nc.dram_tensor("wk", [KLOC * 128, DT * 128], BF, kind="ExternalInput")
    wv = nc.dram_tensor("wv", [128, DT * KLOC * H], BF, kind="ExternalInput")
    wo = nc.dram_tensor("wo", [HEADS * H, D], BF, kind="ExternalInput")
    cosT = nc.dram_tensor("cosT", [H, S], FP32, kind="ExternalInput")
    sinST = nc.dram_tensor("sinST", [H, S], FP32, kind="ExternalInput")
    maskdT = nc.dram_tensor("maskdT", [128, S], BF, kind="ExternalInput")
    out_sh = nc.dram_tensor("out_shard", [S // TP, D], BF, kind="ExternalOutput")

    with tile.TileContext(nc) as tc:
        with tc.tile_pool(name="persist", bufs=1) as persist, \
             tc.tile_pool(name="dram", bufs=1, space="DRAM") as dram:

            qT_t = [persist.tile([128, S], BF, tag=f"qT{i}", name=f"qT{i}")
                    for i in range(HEADS)]
            kT_t = [persist.tile([128, S], BF, tag=f"kT{i}", name=f"kT{i}")
                    for i in range(KLOC)]
            v_t = [persist.tile([128, KLOC * HP], BF, tag=f"v{i}", name=f"v{i}")
                   for i in range(ST)]
            wo_sb = [persist.tile([128, D], BF, tag=f"wo{i}", name=f"wo{i}")
                     for i in range(HEADS)]
            ident = persist.tile([128, 128], BF, tag="ident")
            mdT = persist.tile([128, S], BF, tag="mdT")
            cc_in = [dram.tile([n * 128, D], BF, tag=f"ccin{g}", name=f"cc_in{g}")
                     for g, (st0, n) in enumerate(CC_CHUNKS)]
            cc_out = [dram.tile([n * 32, D], BF, tag=f"ccout{g}", name=f"cc_out{g}")
                      for g, (st0, n) in enumerate(CC_CHUNKS)]

            # ---------------- Phase 1: projections + rope ----------------
            with tc.tile_pool(name="p1", bufs=1) as p1, \
                 tc.tile_pool(name="p1ps", bufs=1, space="PSUM") as p1ps:
                wv_sb = p1.tile([128, DT * KLOC * H], BF, tag="wvsb")

                for q in range(NQ):
                    scol = q * 512
                    # 8 sub-DMAs of 4 d-tiles each; 9 rotating buffers so
                    # the next quarter's first chunk preloads during this
                    # quarter's compute.
                    xqs = [p1.tile([128, 4 * 512], BF, tag="xq", bufs=9,
                                   name=f"xq{q}_{c}") for c in range(8)]

                    def xq(d, a, b):
                        return xqs[d // 4][:, (d % 4) * 512 + a:
                                           (d % 4) * 512 + b]

                    ct = p1.tile([128, 512], FP32, tag="ct", bufs=1,
                                 name=f"ct{q}")
                    st = p1.tile([128, 512], FP32, tag="st", bufs=1,
                                 name=f"st{q}")

                    def wsrc_ap(h):
                        return wq.ap()[h * 128:(h + 1) * 128, :] if h < HEADS \
                            else wk.ap()[(h - HEADS) * 128:(h - HEADS + 1) * 128, :]

                    # weight slabs prefetched one head ahead so their DMAs
                    # queue before the bulk x transfers of the iteration
                    wslab0 = p1.tile([128, DT * 128], BF, tag="wslab", bufs=3,
                                     name=f"wsl{q}_0")
                    nc.sync.dma_start(wslab0[:], wsrc_ap(0))
                    wslab_next = wslab0

                    for h in range(HEADS + KLOC):
                        wslab = wslab_next
                        if h == 0:
                            base = q * (DT * 512)
                            for c in range(8):
                                if q == 0 and c == 0:
                                    # split so the first 2 d-tiles land ASAP
                                    nc.sync.dma_start(
                                        xqs[0][:, :2 * 512],
                                        xP.ap()[:, base: base + 2 * 512])
                                    nc.sync.dma_start(
                                        xqs[0][:, 2 * 512:],
                                        xP.ap()[:, base + 2 * 512:
                                                base + 4 * 512])
                                    continue
                                nc.sync.dma_start(
                                    xqs[c][:],
                                    xP.ap()[:, base + c * 4 * 512:
                                            base + (c + 1) * 4 * 512])
                            nc.sync.dma_start(ct[:], cosT.ap()[:, scol:scol + 512])
                            nc.sync.dma_start(st[:], sinST.ap()[:, scol:scol + 512])
                            if q == 1:
                                nc.sync.dma_start(mdT[:], maskdT.ap())
                            if q == 2:
                                for i in range(HEADS):
                                    nc.sync.dma_start(wo_sb[i][:],
                                                      wo.ap()[i * 128:(i + 1) * 128, :])
                        if h + 1 < HEADS + KLOC:
                            wslab_next = p1.tile([128, DT * 128], BF,
                                                 tag="wslab", bufs=3,
                                                 name=f"wsl{q}_{h + 1}")
                            nc.sync.dma_start(wslab_next[:], wsrc_ap(h + 1))
                        if q == 0 and h == 2:
                            nc.sync.dma_start(wv_sb[:], wv.ap())

                        qp = p1ps.tile([128, 512], FP32, tag="qp", bufs=3)
                        for d in range(DT):
                            nc.tensor.matmul(
                                qp[:],
                                lhsT=wslab[:, d * 128:(d + 1) * 128],
                                rhs=xq(d, 0, 512),
                                start=(d == 0), stop=(d == DT - 1))
                        # rope: out = qp*cos + rot(qp)*sin_signed
                        t1 = p1.tile([128, 512], FP32, tag="t1", bufs=1)
                        nc.vector.tensor_mul(t1[:], qp[:], ct[:])
                        t2 = p1.tile([128, 512], FP32, tag="t2", bufs=1)
                        nc.vector.tensor_mul(t2[0:64, :], qp[64:128, :],
                                             st[0:64, :])
                        nc.vector.tensor_mul(t2[64:128, :], qp[0:64, :],
                                             st[64:128, :])
                        dst = qT_t[h] if h < HEADS else kT_t[h - HEADS]
                        nc.vector.tensor_add(dst[:, scol:scol + 512], t1[:], t2[:])

                    # v projection for the quarter's 4 s-tiles
                    for stl in range(4):
                        sti = q * 4 + stl
                        vp = p1ps.tile([128, KLOC * H], FP32, tag="vp", bufs=2)
                        for d in range(DT):
                            nc.tensor.matmul(
                                vp[:],
                                lhsT=xq(d, stl * 128, (stl + 1) * 128),
                                rhs=wv_sb[:, d * KLOC * H:(d + 1) * KLOC * H],
                                start=(d == 0), stop=(d == DT - 1))
                        vdst = v_t[sti][:].rearrange("p (k c) -> p k c", k=KLOC)
                        nc.scalar.copy(vdst[:, :, 0:H],
                                       vp[:].rearrange("p (k c) -> p k c", k=KLOC))
                        nc.vector.memset(vdst[:, :, H:HP], 1.0)

                make_identity(nc, ident[:])

            # ---------------- Phase 2: attention + out-proj ----------------
            with tc.tile_pool(name="p2", bufs=1) as p2, \
                 tc.tile_pool(name="p2ps", bufs=1, space="PSUM") as p2ps:

                def fire_rs(cg, cs, cn):
                    nc.gpsimd.collective_compute(
                        "ReduceScatter", mybir.AluOpType.add,
                        replica_groups=[[0, 1, 2, 3], [4, 5, 6, 7]],
                        ins=[cc_in[cg].opt()], outs=[cc_out[cg].opt()])
                    nc.sync.dma_start(
                        out_sh.ap()[cs * 32:(cs + cn) * 32, :],
                        cc_out[cg][:])

                def emit_outproj(t0p, yT_list, c, pool=None, tag="op",
                                 bufs=1):
                    """Out-proj dc-chunk c (= ii*8+dc) of segment t0p; fires
                    the ReduceScatter for each cc chunk as it completes."""
                    ii, dc = c // 8, c % 8
                    i = t0p + ii
                    cg = next(ci for ci, (cs, cn) in enumerate(CC_CHUNKS)
                              if cs <= i < cs + cn)
                    c_start, c_n = CC_CHUNKS[cg]
                    op = (pool or p2ps).tile([128, 512], FP32, tag=tag,
                                             bufs=bufs)
                    for hh in range(HEADS):
                        nc.tensor.matmul(
                            op[:],
                            lhsT=yT_list[hh][:, ii * 128:(ii + 1) * 128],
                            rhs=wo_sb[hh][:, dc * 512:(dc + 1) * 512],
                            start=(hh == 0), stop=(hh == HEADS - 1))
                    oev = p2.tile([128, 512], BF, tag="oev", bufs=16)
                    if dc % 2 == 0:
                        nc.vector.tensor_copy(oev[:], op[:])
                    else:
                        nc.scalar.copy(oev[:], op[:])
                    if i == SPLIT_TILE:  # final tile: column-split halves
                        half, hc = dc // 4, dc % 4
                        nc.sync.dma_start(
                            cc_in[cg + half][0:128, hc * 512:(hc + 1) * 512],
                            oev[:])
                        if dc == 3:
                            fire_rs(cg, c_start, c_n)
                        elif dc == 7:
                            fire_rs(cg + 1, c_start, c_n)
                    else:
                        nc.sync.dma_start(
                            cc_in[cg][(i - c_start) * 128:
                                      (i - c_start + 1) * 128,
                                      dc * 512:(dc + 1) * 512],
                            oev[:])
                        if i == c_start + c_n - 1 and dc == 7:
                            fire_rs(cg, c_start, c_n)

                prev = None  # (t0, yT list) of the previous segment
                for t0, nt in SEGMENTS:
                    nj = t0 + nt
                    W = nt * 128
                    yT_sb = [p2.tile([128, W], BF, tag=f"yt{h}", bufs=2,
                                     name=f"yt{t0}_{h}")
                             for h in range(HEADS)]
                    with tc.tile_pool(name=f"seg{t0}", bufs=1) as sgp, \
                         tc.tile_pool(name=f"segps{t0}", bufs=1,
                                      space="PSUM") as sps:
                        es = {}
                        yaug = {}

                        def s_step(h, j):
                            kv = h % KLOC
                            wj = (nt if j <= t0 else t0 + nt - j) * 128
                            ioff = max(t0, j) * 128
                            sp = sps.tile([128, 512], FP32, tag="sp",
                                          bufs=2)
                            nc.tensor.matmul(
                                sp[:, :wj],
                                lhsT=kT_t[kv][:, j * 128:(j + 1) * 128],
                                rhs=qT_t[h][:, ioff:ioff + wj],
                                start=True, stop=True)
                            if j >= t0:  # diagonal block (i == j)
                                nc.vector.tensor_add(
                                    sp[:, 0:128], sp[:, 0:128],
                                    mdT[:, j * 128:(j + 1) * 128])
                            e = sgp.tile([128, W], BF, tag="es",
                                         bufs=2 * nj,
                                         name=f"es{t0}_{h}_{j}")
                            nc.scalar.activation(
                                e[:, :wj], sp[:, :wj],
                                mybir.ActivationFunctionType.Exp)
                            es[(h, j)] = e

                        def y_step(h, j):
                            kv = h % KLOC
                            for ii in range(nt):
                                i = t0 + ii
                                if j > i:
                                    continue
                                if j == 0:
                                    yaug[h][ii] = sps.tile(
                                        [128, 512], FP32, tag="yaug",
                                        bufs=4, name=f"ya{t0}_{h}_{ii}")
                                off = (i - max(t0, j)) * 128
                                nc.tensor.matmul(
                                    yaug[h][ii][:, 0:HP],
                                    lhsT=es[(h, j)][:, off:off + 128],
                                    rhs=v_t[j][:, kv * HP:(kv + 1) * HP],
                                    start=(j == 0), stop=(j == i))

                        def emit_norm_transpose(h):
                            for ii in range(nt):
                                yp = yaug[h][ii]
                                rc = p2.tile([128, 1], FP32, tag="rc", bufs=4)
                                nc.vector.reciprocal(rc[:], yp[:, H:HP])
                                yn = p2.tile([128, 128], BF, tag="yn", bufs=3)
                                nc.vector.tensor_scalar_mul(yn[:], yp[:, 0:H],
                                                            rc[:])
                                tp = sps.tile([128, 128], FP32, tag="tp",
                                              bufs=1)
                                nc.tensor.matmul(tp[:], lhsT=yn[:],
                                                 rhs=ident[:],
                                                 start=True, stop=True)
                                nc.vector.tensor_copy(
                                    yT_sb[h][:, ii * 128:(ii + 1) * 128], tp[:])

                        # strip-interleaved pipeline: scores for head h+1
                        # alternate with y accumulation for head h at strip
                        # granularity; the previous segment's out-projection
                        # rides along (a few dc-chunks per head iteration, a
                        # couple even during priming) so exp (ACT) gets
                        # PE-runway and RS chunks fire early.
                        op_sched = [4, 6, 6, 5, 4, 3, 2, 1, 1]
                        op_next = 0

                        def emit_op_filler(idx):
                            nonlocal op_next
                            if prev is None:
                                return
                            for c in range(op_next,
                                           op_next + op_sched[idx]):
                                emit_outproj(prev[0], prev[1], c)
                            op_next += op_sched[idx]

                        for j in range(nj):
                            s_step(0, j)
                            if j == nj // 2:
                                emit_op_filler(0)
                        for h in range(HEADS):
                            if h > 0:
                                emit_norm_transpose(h - 1)
                            yaug[h] = [None] * nt
                            for j in range(nj):
                                if h + 1 < HEADS:
                                    s_step(h + 1, j)
                                y_step(h, j)
                            emit_op_filler(h + 1)
                        emit_norm_transpose(HEADS - 1)
                    prev = (t0, yT_sb)

                # final segment's out-projection (deeper PSUM buffering now
                # that the attention banks are free)
                with tc.tile_pool(name="fps", bufs=1, space="PSUM") as fps:
                    for c in range(8 * SEGMENTS[-1][1]):
                        emit_outproj(prev[0], prev[1], c, pool=fps,
                                     tag="opf", bufs=3)

    nc.compile()
    return nc


# ---------------------------------------------------------------------------
# Fallback (non-causal mask) — original baseline implementation.
# ---------------------------------------------------------------------------

def _build_fallback():
    NG = ST // 4
    FB_CHUNKS = [(0, 2), (2, 2), (4, 2), (6, 2), (8, 2), (10, 2), (12, 2),
                 (14, 1), (15, 1)]
    nc = bacc.Bacc("TRN2", target_bir_lowering=False, debug=False,
                   enable_asserts=False, num_devices=N_CORES)

    xP = nc.dram_tensor("xP", [128, 2 * DT * (S // 2)], BF, kind="ExternalInput")
    wq = nc.dram_tensor("wq", [HEADS * 128, DT * 128], BF, kind="ExternalInput")
    wk = nc.dram_tensor("wk", [KLOC * 128, DT * 128], BF, kind="ExternalInput")
    wv = nc.dram_tensor("wv", [128, DT * KLOC * H], BF, kind="ExternalInput")
    wo = nc.dram_tensor("wo", [HEADS * H, D], BF, kind="ExternalInput")
    cosT = nc.dram_tensor("cosT", [H, S], FP32, kind="ExternalInput")
    sinST = nc.dram_tensor("sinST", [H, S], FP32, kind="ExternalInput")
    maskf = nc.dram_tensor("maskf", [S, S], FP32, kind="ExternalInput")
    out_sh = nc.dram_tensor("out_shard", [S // TP, D], BF, kind="ExternalOutput")

    with tile.TileContext(nc) as tc:
        with tc.tile_pool(name="persist", bufs=1) as persist, \
             tc.tile_pool(name="dram", bufs=1, space="DRAM") as dram:

            kT_t = [persist.tile([128, S], BF, tag=f"kT{i}", name=f"kT{i}")
                    for i in range(KLOC)]
            v_t = [persist.tile([128, KLOC * H], BF, tag=f"v{i}", name=f"v{i}")
                   for i in range(ST)]
            wo_sb = [persist.tile([128, D], BF, tag=f"wo{i}", name=f"wo{i}")
                     for i in range(HEADS)]
            qT_dram = dram.tile([HEADS * 128, S], BF, tag="qtd", name="qT_dram")
            cc_in = [dram.tile([n * 128, D], BF, tag=f"ccin{g}", name=f"cc_in{g}")
                     for g, (st0, n) in enumerate(FB_CHUNKS)]
            cc_out = [dram.tile([n * 32, D], BF, tag=f"ccout{g}", name=f"cc_out{g}")
                      for g, (st0, n) in enumerate(FB_CHUNKS)]

            with tc.tile_pool(name="p1", bufs=1) as p1, \
                 tc.tile_pool(name="p1ps", bufs=1, space="PSUM") as p1ps:
                ct = p1.tile([H, S], FP32, tag="ct")
                st = p1.tile([H, S], FP32, tag="st")
                wv_sb = p1.tile([128, DT * KLOC * H], BF, tag="wvsb")

                for half in range(2):
                    scols = (half * (S // 2), (half + 1) * (S // 2))
                    xth_t = [p1.tile([128, 8 * (S // 2)], BF, tag="xth", bufs=4,
                                     name=f"xth{half}_{qq}") for qq in range(4)]

                    def xth_dma(qq, split=False):
                        base = (half * DT + qq * 8) * (S // 2)
                        if split:
                            hw_ = 4 * (S // 2)
                            nc.sync.dma_start(xth_t[qq][:, :hw_],
                                              xP.ap()[:, base: base + hw_])
                            nc.sync.dma_start(xth_t[qq][:, hw_:],
                                              xP.ap()[:, base + hw_: base + 8 * (S // 2)])
                        else:
                            nc.sync.dma_start(
                                xth_t[qq][:],
                                xP.ap()[:, base: base + 8 * (S // 2)])

                    if half == 1:
                        for qq in range(4):
                            xth_dma(qq)

                    def xth(d, a, b):
                        return xth_t[d // 8][:, (d % 8) * (S // 2) + a:
                                             (d % 8) * (S // 2) + b]

                    for h in range(HEADS + KLOC):
                        wsrc = wq.ap()[h * 128:(h + 1) * 128, :] if h < HEADS \
                            else wk.ap()[(h - HEADS) * 128:(h - HEADS + 1) * 128, :]
                        if half == 0 and h == 0:
                            xth_dma(0, split=True)
                        wslab = p1.tile([128, DT * 128], BF, tag="wslab", bufs=2)
                        nc.sync.dma_start(wslab[:], wsrc)
                        if half == 0 and h == 0:
                            nc.sync.dma_start(ct[:], cosT.ap())
                            nc.sync.dma_start(st[:], sinST.ap())
                            for qq in range(1, 4):
                                xth_dma(qq)
                            nc.sync.dma_start(wv_sb[:], wv.ap())
                        for sc in range(2):
                            lo = sc * 512
                            qp = p1ps.tile([128, 512], FP32, tag="qp", bufs=3)
                            for d in range(DT):
                                nc.tensor.matmul(
                                    qp[:],
                                    lhsT=wslab[:, d * 128:(d + 1) * 128],
                                    rhs=xth(d, lo, lo + 512),
                                    start=(d == 0), stop=(d == DT - 1))
                            gcol = scols[0] + lo
                            t1 = p1.tile([128, 512], FP32, tag="t1", bufs=2)
                            nc.vector.tensor_mul(t1[:], qp[:], ct[:, gcol:gcol + 512])
                            t2 = p1.tile([128, 512], FP32, tag="t2", bufs=2)
                            nc.vector.tensor_mul(t2[0:64, :], qp[64:128, :],
                                                 st[0:64, gcol:gcol + 512])
                            nc.vector.tensor_mul(t2[64:128, :], qp[0:64, :],
                                                 st[64:128, gcol:gcol + 512])
                            if h < HEADS:
                                robf = p1.tile([128, 512], BF, tag="robf", bufs=2)
                                nc.vector.tensor_add(robf[:], t1[:], t2[:])
                                nc.sync.dma_start(
                                    qT_dram[h * 128:(h + 1) * 128, gcol:gcol + 512],
                                    robf[:])
                            else:
                                nc.vector.tensor_add(
                                    kT_t[h - HEADS][:, gcol:gcol + 512], t1[:], t2[:])

                    for stl in range(ST // 2):
                        sti = half * (ST // 2) + stl
                        vp = p1ps.tile([128, KLOC * H], FP32, tag="vp", bufs=2)
                        for d in range(DT):
                            nc.tensor.matmul(
                                vp[:],
                                lhsT=xth(d, stl * 128, (stl + 1) * 128),
                                rhs=wv_sb[:, d * KLOC * H:(d + 1) * KLOC * H],
                                start=(d == 0), stop=(d == DT - 1))
                        nc.scalar.copy(v_t[sti][:], vp[:])

                for i in range(HEADS):
                    nc.sync.dma_start(wo_sb[i][:], wo.ap()[i * 128:(i + 1) * 128, :])

            with tc.tile_pool(name="p2", bufs=1) as p2, \
                 tc.tile_pool(name="p2ps", bufs=1, space="PSUM") as p2ps:
                ident = p2.tile([128, 128], BF, tag="ident")
                make_identity(nc, ident[:])

                qg_all = [p2.tile([128, S], BF, tag=f"qga{h}", name=f"qga{h}")
                          for h in range(HEADS)]
                for h in range(HEADS):
                    nc.sync.dma_start(qg_all[h][:], qT_dram[h * 128:(h + 1) * 128, :])

                segs = [(0, 4), (4, 4), (8, 4), (12, 4)]
                for t0, nt in segs:
                    W = nt * 128
                    mrow = [p2.tile([128, S], FP32, tag="mrow", bufs=4,
                                    name=f"mrow{t0}_{it}") for it in range(nt)]
                    for it in range(nt):
                        i = t0 + it
                        nc.sync.dma_start(mrow[it][:], maskf.ap()[i * 128:(i + 1) * 128, :])

                    yT_sb = [p2.tile([128, W], BF, tag=f"yt{h}", bufs=2,
                                     name=f"yt{t0}_{h}") for h in range(HEADS)]
                    for h in range(HEADS):
                        kv = h % KLOC
                        nquad = NG
                        pTq = [p2.tile([128, 4 * W], BF, tag=f"ptq{q}", bufs=1,
                                       name=f"ptq{t0}_{h}_{q}") for q in range(nquad)]
                        for it in range(nt):
                            i = t0 + it
                            nsk = S
                            prow = p2.tile([128, S], BF, tag="prow", bufs=3)
                            sums = []
                            nch = (nsk + 1023) // 1024
                            for c in range(nch):
                                w = min(1024, nsk - c * 1024)
                                sp = p2ps.tile([128, 1024], FP32, tag="sp", bufs=2)
                                for cc in range((w + 511) // 512):
                                    ww = min(512, w - cc * 512)
                                    o = cc * 512
                                    nc.tensor.matmul(
                                        sp[:, o:o + ww],
                                        lhsT=qg_all[h][:, i * 128:(i + 1) * 128],
                                        rhs=kT_t[kv][:, c * 1024 + o: c * 1024 + o + ww],
                                        start=True, stop=True)
                                nc.vector.tensor_add(
                                    sp[:, :w], sp[:, :w],
                                    mrow[it][:, c * 1024: c * 1024 + w])
                                sm = p2.tile([128, 1], FP32, tag="sm", bufs=8)
                                nc.scalar.activation(
                                    prow[:, c * 1024: c * 1024 + w], sp[:, :w],
                                    mybir.ActivationFunctionType.Exp, accum_out=sm[:])
                                sums.append(sm)
                            if nch == 2:
                                tot = p2.tile([128, 1], FP32, tag="tot", bufs=4)
                                nc.vector.tensor_add(tot[:], sums[0][:], sums[1][:])
                            else:
                                tot = sums[0]
                            rc = p2.tile([128, 1], FP32, tag="rc", bufs=4)
                            nc.vector.reciprocal(rc[:], tot[:])
                            diag = p2.tile([128, 128], BF, tag="diag", bufs=4)
                            nc.vector.tensor_scalar_mul(diag[:], ident[:], rc[:])
                            jtop = ST - 1
                            for qd in range(jtop // 4 + 1):
                                jlo, jhi = 4 * qd, min(4 * qd + 3, jtop)
                                nq = jhi - jlo + 1
                                tpp = p2ps.tile([128, 512], FP32, tag="tp", bufs=2)
                                for j in range(jlo, jhi + 1):
                                    nc.tensor.matmul(
                                        tpp[:, (j - jlo) * 128:(j - jlo + 1) * 128],
                                        lhsT=prow[:, j * 128:(j + 1) * 128],
                                        rhs=diag[:], start=True, stop=True)
                                pt_dst = pTq[qd][:].rearrange("p (a b) -> p a b", a=4)[
                                    :, 0:nq, it * 128:(it + 1) * 128]
                                pt_src = tpp[:, :nq * 128].rearrange(
                                    "p (a b) -> p a b", b=128)
                                if (it + qd) % 2:
                                    nc.scalar.copy(pt_dst, pt_src)
                                else:
                                    nc.vector.tensor_copy(pt_dst, pt_src)
                        yp = p2ps.tile([128, W], FP32, tag="yp", bufs=1)
                        jmax = ST
                        for j in range(jmax):
                            lo = 0
                            nc.tensor.matmul(
                                yp[:, lo:W],
                                lhsT=v_t[j][:, kv * H:(kv + 1) * H],
                                rhs=pTq[j // 4][:, (j % 4) * W + lo: (j % 4) * W + W],
                                start=(j == 0), stop=(j == jmax - 1))
                        nc.scalar.copy(yT_sb[h][:], yp[:])

                    for it in range(nt):
                        i = t0 + it
                        cg = next(ci for ci, (cs, cn) in enumerate(FB_CHUNKS)
                                  if cs <= i < cs + cn)
                        c_start, c_n = FB_CHUNKS[cg]
                        for dc in range(8):
                            op = p2ps.tile([128, 512], FP32, tag="op", bufs=1)
                            for hh in range(HEADS):
                                nc.tensor.matmul(
                                    op[:],
                                    lhsT=yT_sb[hh][:, it * 128:(it + 1) * 128],
                                    rhs=wo_sb[hh][:, dc * 512:(dc + 1) * 512],
                                    start=(hh == 0), stop=(hh == HEADS - 1))
                            oev = p2.tile([128, 512], BF, tag="oev", bufs=16)
                            nc.scalar.copy(oev[:], op[:])
                            nc.sync.dma_start(
                                cc_in[cg][(i - c_start) * 128:(i - c_start + 1) * 128,
                                          dc * 512:(dc + 1) * 512],
                                oev[:])
                        if i == c_start + c_n - 1:
                            nc.gpsimd.collective_compute(
                                "ReduceScatter", mybir.AluOpType.add,
                                replica_groups=[[0, 1, 2, 3], [4, 5, 6, 7]],
                                ins=[cc_in[cg].opt()], outs=[cc_out[cg].opt()])
                            orow = sum(cn * 32 for cs, cn in FB_CHUNKS[:cg])
                            nc.sync.dma_start(
                                out_sh.ap()[orow: orow + c_n * 32, :], cc_out[cg][:])

    nc.compile()
    return nc


_CANON_MASK = None


def _is_causal(mask: np.ndarray) -> bool:
    global _CANON_MASK
    if _CANON_MASK is None:
        _CANON_MASK = np.triu(np.full((S, S), -1e9, dtype=np.float32), k=1)
    return mask.shape == (S, S) and np.array_equal(mask, _CANON_MASK)


def _prepare(x, wq, wk, wv, wo, mask, sin, cos):
    causal = _is_causal(np.asarray(mask, dtype=np.float32))
    if causal not in _CACHE:
        _CACHE[causal] = _build_fast() if causal else _build_fallback()
    nc = _CACHE[causal]

    x = np.asarray(x, dtype=np.float32)
    scale = np.float32(H ** -0.5)
    cosT = np.ascontiguousarray(np.asarray(cos, np.float32).T)          # [H, S]
    sinT = np.asarray(sin, np.float32).T.copy()                          # [H, S]
    sinT[0:H // 2] = -sinT[0:H // 2]                                     # signed
    # per-core weight shards; head order = r-major over local kv heads
    in_maps = []
    for c in range(N_CORES):
        b, tp = c // TP, c % TP
        ks = slice(tp * KLOC, (tp + 1) * KLOC)
        wq_c = np.asarray(wq, np.float32)[:, :, ks, :].reshape(D, HEADS * H)
        wk_c = (np.asarray(wk, np.float32)[:, ks, :] * scale).reshape(D, KLOC * H)
        wv_c = np.asarray(wv, np.float32)[:, ks, :].reshape(D, KLOC * H)
        if causal:
            xp = x[b].reshape(NQ, 512, DT, 128).transpose(3, 0, 2, 1) \
                     .reshape(128, NQ * DT * 512).astype(BF16)
        else:
            xp = x[b].reshape(2, S // 2, DT, 128).transpose(3, 0, 2, 1) \
                     .reshape(128, 2 * DT * (S // 2)).astype(BF16)
        m = {
            "xP": xp,
            "wq": wq_c.reshape(DT, 128, HEADS, H).transpose(2, 1, 0, 3)
                      .reshape(HEADS * 128, DT * 128).astype(BF16),
            "wk": wk_c.reshape(DT, 128, KLOC, H).transpose(2, 1, 0, 3)
                      .reshape(KLOC * 128, DT * 128).astype(BF16),
            "wv": wv_c.reshape(DT, 128, KLOC * H).transpose(1, 0, 2)
                      .reshape(128, DT * KLOC * H).astype(BF16),
            "wo": np.asarray(wo, np.float32)[:, ks, :, :].reshape(HEADS * H, D).astype(BF16),
            "cosT": cosT,
            "sinST": sinT,
        }
        if causal:
            md = np.empty((128, S), np.float32)
            for i in range(ST):
                md[:, i * 128:(i + 1) * 128] = mask[i * 128:(i + 1) * 128,
                                                    i * 128:(i + 1) * 128].T
            m["maskdT"] = md.astype(BF16)
        else:
            m["maskf"] = np.asarray(mask, np.float32)
        in_maps.append(m)
    return nc, in_maps, causal


def _assemble(results, causal):
    chunks = CC_CHUNKS if causal else \
        [(0, 2), (2, 2), (4, 2), (6, 2), (8, 2), (10, 2), (12, 2), (14, 1), (15, 1)]
    out = np.empty((B, S, D), dtype=np.float32)
    for c in range(N_CORES):
        b, tp = c // TP, c % TP
        sh = results[c]["out_shard"].astype(np.float32)
        if causal:
            for cs, cn in chunks:
                rows = cn * 32
                out[b, cs * 128 + tp * rows: cs * 128 + (tp + 1) * rows, :] = \
                    sh[cs * 32: cs * 32 + rows]
        else:
            orow = 0
            for cs, cn in chunks:
                rows = cn * 32
                out[b, cs * 128 + tp * rows: cs * 128 + (tp + 1) * rows, :] = \
                    sh[orow: orow + rows]
                orow += rows
    return out


def kernel(x, wq, wk, wv, wo, mask, sin, cos):
    nc, in_maps, causal = _prepare(x, wq, wk, wv, wo, mask, sin, cos)
    try:
        res = bass_utils.run_bass_kernel_spmd(nc, in_maps,
                                              core_ids=list(range(N_CORES)))
    except Exception:
        # transient device-side failures (e.g. NRT exec-unit errors) have
        # been observed once; a clean re-run succeeds.
        import time as _time
        _time.sleep(2.0)
        res = bass_utils.run_bass_kernel_spmd(nc, in_maps,
                                              core_ids=list(range(N_CORES)))
    return _assemble(res.results, causal)


def _traced_run(x, wq, wk, wv, wo, mask, sin, cos):
    """Like kernel() but with NTFF tracing; returns BassKernelResults."""
    nc, in_maps, causal = _prepare(x, wq, wk, wv, wo, mask, sin, cos)
    res = bass_utils.run_bass_kernel_spmd(nc, in_maps, core_ids=list(range(N_CORES)),
                                          trace=True)
    res.full_output = _assemble(res.results, causal)
    return res


# revision 34
# speedup vs baseline: 1.0168x; 1.0168x over previous
"""Tensor-parallel fused attention kernel for Trainium2 (8 NeuronCores).

Sharding: DP=2 over batch x TP=4 over kv-head pairs. Each core computes
q/k/v projections + RoPE + causal attention + output projection for its
(batch, 2 kv heads) shard in bf16, then a 4-core ReduceScatter combines
the partial output projections; the host assembles the disjoint row
shards into the full [2, 2048, 4096] output.

Causal fast path computes attention scores transposed (S^T = K^T Q per
block, partition dim = key position): y = expS^T . [V|1] with a ones
column appended to V so the softmax denominator falls out of the same
matmul. This removes the per-block P transposes and the exp-sum/
reciprocal/diag chain from the PE critical path; only one small
[i,H]->[H,i] transpose per output tile remains before the out-proj.
"""
import sys

for _p in ("/opt/trn_rl_repo", "/root/.axon_site/_ro/trn_rl_repo"):
    if _p not in sys.path:
        sys.path.append(_p)

import math
import numpy as np
import ml_dtypes

import concourse.bass as bass
import concourse.mybir as mybir
import concourse.tile as tile
from concourse import bacc
from concourse import bass_utils
from concourse.masks import make_identity

BF16 = ml_dtypes.bfloat16
FP32 = mybir.dt.float32
BF = mybir.dt.bfloat16

B, S, D = 2, 2048, 4096
R, K, H = 4, 8, 128
N_CORES = 8
TP = 4            # tensor-parallel ways (kv-head axis)
KLOC = K // TP    # kv heads per core = 2
HEADS = R * KLOC  # query heads per core = 8
DT = D // 128     # 32 d-tiles
ST = S // 128     # 16 s-tiles
HP = H + 1        # v columns per kv head incl. ones column
NQ = 4            # x streamed in 4 quarters of 512 sequence cols

# 4-tile attention segments; each segment's out-projection is emitted
# interleaved into the NEXT segment's head loop so exp (ACT) overlaps the
# out-proj matmuls and the ReduceScatter chunks fire a segment early.
# Order: (0,4) first (cheap attention, PE-bound start), then (12,4) whose
# heavy exp load hides under seg(0,4)'s interleaved out-proj; last-processed
# rows 8-11 use single-tile chunks, the final one column-split, to minimize
# the serial RS tail.
SEGMENTS = [(0, 4), (12, 4), (4, 4), (8, 4)]
# ReduceScatter chunks (start_tile, n_tiles); fired as soon as the
# out-projection rows for the chunk are complete. Two-tile chunks
# minimize total CC time (the end chain is CC-throughput-bound).
CC_CHUNKS = [(0, 2), (2, 2), (12, 2), (14, 2), (4, 2), (6, 2), (8, 2),
             (10, 2)]
SPLIT_TILE = -1  # column-split of the final chunk disabled

_CACHE = {}


def _build_fast():
    """Causal-mask fast path."""
    nc = bacc.Bacc("TRN2", target_bir_lowering=False, debug=False,
                   enable_asserts=False, num_devices=N_CORES)

    # x quarter-major: col = q*(DT*512) + d*512 + s_local
    xP = nc.dram_tensor("xP", [128, NQ * DT * 512], BF, kind="ExternalInput")
    wq = nc.dram_tensor("wq", [HEADS * 128, DT * 128], BF, kind="ExternalInput")
    wk = nc.dram_tensor("wk", [KLOC * 128, DT * 128], BF, kind="ExternalInput")
    wv = nc.dram_tensor("wv", [128, DT * KLOC * H], BF, kind="ExternalInput")
    wo = nc.dram_tensor("wo", [HEADS * H, D], BF, kind="ExternalInput")
    cosT = nc.dram_tensor("cosT", [H, S], FP32, kind="ExternalInput")
    sinST = nc.dram_tensor("sinST", [H, S], FP32, kind="ExternalInput")
    maskdT = nc.dram_tensor("maskdT", [128, S], BF, kind="ExternalInput")
    out_sh = nc.dram_tensor("out_shard", [S // TP, D], BF, kind="ExternalOutput")

    with tile.TileContext(nc) as tc:
        with tc.tile_pool(name="persist", bufs=1) as persist, \
             tc.tile_pool(name="dram", bufs=1, space="DRAM") as dram:

            qT_t = [persist.tile([128, S], BF, tag=f"qT{i}", name=f"qT{i}")
                    for i in range(HEADS)]
            kT_t = [persist.tile([128, S], BF, tag=f"kT{i}", name=f"kT{i}")
                    for i in range(KLOC)]
            v_t = [persist.tile([128, KLOC * HP], BF, tag=f"v{i}", name=f"v{i}")
                   for i in range(ST)]
            wo_sb = [persist.tile([128, D], BF, tag=f"wo{i}", name=f"wo{i}")
                     for i in range(HEADS)]
            ident = persist.tile([128, 128], BF, tag="ident")
            mdT = persist.tile([128, S], BF, tag="mdT")
            cc_in = [dram.tile([n * 128, D], BF, tag=f"ccin{g}", name=f"cc_in{g}")
                     for g, (st0, n) in enumerate(CC_CHUNKS)]
            cc_out = [dram.tile([n * 32, D], BF, tag=f"ccout{g}", name=f"cc_out{g}")
                      for g, (st0, n) in enumerate(CC_CHUNKS)]

            # ---------------- Phase 1: projections + rope ----------------
            with tc.tile_pool(name="p1", bufs=1) as p1, \
                 tc.tile_pool(name="p1ps", bufs=1, space="PSUM") as p1ps:
                wv_sb = p1.tile([128, DT * KLOC * H], BF, tag="wvsb")

                for q in range(NQ):
                    scol = q * 512
                    # 8 sub-DMAs of 4 d-tiles each; 9 rotating buffers so
                    # the next quarter's first chunk preloads during this
                    # quarter's compute.
                    xqs = [p1.tile([128, 4 * 512], BF, tag="xq", bufs=9,
                                   name=f"xq{q}_{c}") for c in range(8)]

                    def xq(d, a, b):
                        return xqs[d // 4][:, (d % 4) * 512 + a:
                                           (d % 4) * 512 + b]

                    ct = p1.tile([128, 512], FP32, tag="ct", bufs=1,
                                 name=f"ct{q}")
                    st = p1.tile([128, 512], FP32, tag="st", bufs=1,
                                 name=f"st{q}")

                    def wsrc_ap(h):
                        return wq.ap()[h * 128:(h + 1) * 128, :] if h < HEADS \
                            else wk.ap()[(h - HEADS) * 128:(h - HEADS + 1) * 128, :]

                    # weight slabs prefetched one head ahead so their DMAs
                    # queue before the bulk x transfers of the iteration
                    wslab0 = p1.tile([128, DT * 128], BF, tag="wslab", bufs=3,
                                     name=f"wsl{q}_0")
                    nc.sync.dma_start(wslab0[:], wsrc_ap(0))
                    wslab_next = wslab0

                    for h in range(HEADS + KLOC):
                        wslab = wslab_next
                        if h == 0:
                            base = q * (DT * 512)
                            for c in range(8):
                                if q == 0 and c == 0:
                                    # split so the first 2 d-tiles land ASAP
                                    nc.sync.dma_start(
                                        xqs[0][:, :2 * 512],
                                        xP.ap()[:, base: base + 2 * 512])
                                    nc.sync.dma_start(
                                        xqs[0][:, 2 * 512:],
                                        xP.ap()[:, base + 2 * 512:
                                                base + 4 * 512])
                                    continue
                                nc.sync.dma_start(
                                    xqs[c][:],
                                    xP.ap()[:, base + c * 4 * 512:
                                            base + (c + 1) * 4 * 512])
                            nc.sync.dma_start(ct[:], cosT.ap()[:, scol:scol + 512])
                            nc.sync.dma_start(st[:], sinST.ap()[:, scol:scol + 512])
                            if q == 1:
                                nc.sync.dma_start(mdT[:], maskdT.ap())
                            if q == 2:
                                for i in range(HEADS):
                                    nc.sync.dma_start(wo_sb[i][:],
                                                      wo.ap()[i * 128:(i + 1) * 128, :])
                        if h + 1 < HEADS + KLOC:
                            wslab_next = p1.tile([128, DT * 128], BF,
                                                 tag="wslab", bufs=3,
                                                 name=f"wsl{q}_{h + 1}")
                            nc.sync.dma_start(wslab_next[:], wsrc_ap(h + 1))
                        if q == 0 and h == 2:
                            nc.sync.dma_start(wv_sb[:], wv.ap())

                        qp = p1ps.tile([128, 512], FP32, tag="qp", bufs=3)
                        for d in range(DT):
                            nc.tensor.matmul(
                                qp[:],
                                lhsT=wslab[:, d * 128:(d + 1) * 128],
                                rhs=xq(d, 0, 512),
                                start=(d == 0), stop=(d == DT - 1))
                        # rope: out = qp*cos + rot(qp)*sin_signed
                        t1 = p1.tile([128, 512], FP32, tag="t1", bufs=1)
                        nc.vector.tensor_mul(t1[:], qp[:], ct[:])
                        t2 = p1.tile([128, 512], FP32, tag="t2", bufs=1)
                        nc.vector.tensor_mul(t2[0:64, :], qp[64:128, :],
                                             st[0:64, :])
                        nc.vector.tensor_mul(t2[64:128, :], qp[0:64, :],
                                             st[64:128, :])
                        dst = qT_t[h] if h < HEADS else kT_t[h - HEADS]
                        nc.vector.tensor_add(dst[:, scol:scol + 512], t1[:], t2[:])

                    # v projection for the quarter's 4 s-tiles
                    for stl in range(4):
                        sti = q * 4 + stl
                        vp = p1ps.tile([128, KLOC * H], FP32, tag="vp", bufs=2)
                        for d in range(DT):
                            nc.tensor.matmul(
                                vp[:],
                                lhsT=xq(d, stl * 128, (stl + 1) * 128),
                                rhs=wv_sb[:, d * KLOC * H:(d + 1) * KLOC * H],
                                start=(d == 0), stop=(d == DT - 1))
                        vdst = v_t[sti][:].rearrange("p (k c) -> p k c", k=KLOC)
                        nc.scalar.copy(vdst[:, :, 0:H],
                                       vp[:].rearrange("p (k c) -> p k c", k=KLOC))
                        nc.vector.memset(vdst[:, :, H:HP], 1.0)

                make_identity(nc, ident[:])

            # ---------------- Phase 2: attention + out-proj ----------------
            with tc.tile_pool(name="p2", bufs=1) as p2, \
                 tc.tile_pool(name="p2ps", bufs=1, space="PSUM") as p2ps:

                def fire_rs(cg, cs, cn):
                    nc.gpsimd.collective_compute(
                        "ReduceScatter", mybir.AluOpType.add,
                        replica_groups=[[0, 1, 2, 3], [4, 5, 6, 7]],
                        ins=[cc_in[cg].opt()], outs=[cc_out[cg].opt()])
                    nc.sync.dma_start(
                        out_sh.ap()[cs * 32:(cs + cn) * 32, :],
                        cc_out[cg][:])

                def emit_outproj(t0p, yT_list, c, pool=None, tag="op",
                                 bufs=1):
                    """Out-proj dc-chunk c (= ii*8+dc) of segment t0p; fires
                    the ReduceScatter for each cc chunk as it completes."""
                    ii, dc = c // 8, c % 8
                    i = t0p + ii
                    cg = next(ci for ci, (cs, cn) in enumerate(CC_CHUNKS)
                              if cs <= i < cs + cn)
                    c_start, c_n = CC_CHUNKS[cg]
                    op = (pool or p2ps).tile([128, 512], FP32, tag=tag,
                                             bufs=bufs)
                    for hh in range(HEADS):
                        nc.tensor.matmul(
                            op[:],
                            lhsT=yT_list[hh][:, ii * 128:(ii + 1) * 128],
                            rhs=wo_sb[hh][:, dc * 512:(dc + 1) * 512],
                            start=(hh == 0), stop=(hh == HEADS - 1))
                    oev = p2.tile([128, 512], BF, tag="oev", bufs=16)
                    if dc % 2 == 0:
                        nc.vector.tensor_copy(oev[:], op[:])
                    else:
                        nc.scalar.copy(oev[:], op[:])
                    if i == SPLIT_TILE:  # final tile: column-split halves
                        half, hc = dc // 4, dc % 4
                        nc.sync.dma_start(
                            cc_in[cg + half][0:128, hc * 512:(hc + 1) * 512],
                            oev[:])
                        if dc == 3:
                            fire_rs(cg, c_start, c_n)
                        elif dc == 7:
                            fire_rs(cg + 1, c_start, c_n)
                    else:
                        nc.sync.dma_start(
                            cc_in[cg][(i - c_start) * 128:
                                      (i - c_start + 1) * 128,
                                      dc * 512:(dc + 1) * 512],
                            oev[:])
                        if i == c_start + c_n - 1 and dc == 7:
                            fire_rs(cg, c_start, c_n)

                prev = None  # (t0, yT list) of the previous segment
                for t0, nt in SEGMENTS:
                    nj = t0 + nt
                    W = nt * 128
                    yT_sb = [p2.tile([128, W], BF, tag=f"yt{h}", bufs=2,
                                     name=f"yt{t0}_{h}")
                             for h in range(HEADS)]
                    with tc.tile_pool(name=f"seg{t0}", bufs=1) as sgp, \
                         tc.tile_pool(name=f"segps{t0}", bufs=1,
                                      space="PSUM") as sps:
                        es = {}
                        yaug = {}

                        def s_step(h, j):
                            kv = h % KLOC
                            wj = (nt if j <= t0 else t0 + nt - j) * 128
                            ioff = max(t0, j) * 128
                            sp = sps.tile([128, 512], FP32, tag="sp",
                                          bufs=2)
                            nc.tensor.matmul(
                                sp[:, :wj],
                                lhsT=kT_t[kv][:, j * 128:(j + 1) * 128],
                                rhs=qT_t[h][:, ioff:ioff + wj],
                                start=True, stop=True)
                            if j >= t0:  # diagonal block (i == j)
                                nc.vector.tensor_add(
                                    sp[:, 0:128], sp[:, 0:128],
                                    mdT[:, j * 128:(j + 1) * 128])
                            e = sgp.tile([128, W], BF, tag="es",
                                         bufs=2 * nj,
                                         name=f"es{t0}_{h}_{j}")
                            nc.scalar.activation(
                                e[:, :wj], sp[:, :wj],
                                mybir.ActivationFunctionType.Exp)
                            es[(h, j)] = e

                        def y_step(h, j):
                            kv = h % KLOC
                            for ii in range(nt):
                                i = t0 + ii
                                if j > i:
                                    continue
                                if j == 0:
                                    yaug[h][ii] = sps.tile(
                                        [128, 512], FP32, tag="yaug",
                                        bufs=4, name=f"ya{t0}_{h}_{ii}")
                                off = (i - max(t0, j)) * 128
                                nc.tensor.matmul(
                                    yaug[h][ii][:, 0:HP],
                                    lhsT=es[(h, j)][:, off:off + 128],
                                    rhs=v_t[j][:, kv * HP:(kv + 1) * HP],
                                    start=(j == 0), stop=(j == i))

                        def emit_norm_transpose(h):
                            for ii in range(nt):
                                yp = yaug[h][ii]
                                rc = p2.tile([128, 1], FP32, tag="rc", bufs=4)
                                nc.vector.reciprocal(rc[:], yp[:, H:HP])
                                yn = p2.tile([128, 128], BF, tag="yn", bufs=3)
                                nc.vector.tensor_scalar_mul(yn[:], yp[:, 0:H],
                                                            rc[:])
                                tp = sps.tile([128, 128], FP32, tag="tp",
                                              bufs=1)
                                nc.tensor.matmul(tp[:], lhsT=yn[:],
                                                 rhs=ident[:],
                                                 start=True, stop=True)
                                nc.vector.tensor_copy(
                                    yT_sb[h][:, ii * 128:(ii + 1) * 128], tp[:])

                        # strip-interleaved pipeline: scores for head h+1
                        # alternate with y accumulation for head h at strip
                        # granularity; the previous segment's out-projection
                        # rides along (a few dc-chunks per head iteration, a
                        # couple even during priming) so exp (ACT) gets
                        # PE-runway and RS chunks fire early.
                        op_sched = [6, 6, 6, 5, 4, 3, 2, 0, 0]
                        op_next = 0

                        def emit_op_filler(idx):
                            nonlocal op_next
                            if prev is None:
                                return
                            for c in range(op_next,
                                           op_next + op_sched[idx]):
                                emit_outproj(prev[0], prev[1], c)
                            op_next += op_sched[idx]

                        for j in range(nj):
                            s_step(0, j)
                            if j == nj // 2:
                                emit_op_filler(0)
                        for h in range(HEADS):
                            if h > 0:
                                emit_norm_transpose(h - 1)
                            yaug[h] = [None] * nt
                            for j in range(nj):
                                if h + 1 < HEADS:
                                    s_step(h + 1, j)
                                y_step(h, j)
                            emit_op_filler(h + 1)
                        emit_norm_transpose(HEADS - 1)
                    prev = (t0, yT_sb)

                # final segment's out-projection (deeper PSUM buffering now
                # that the attention banks are free)
                with tc.tile_pool(name="fps", bufs=1, space="PSUM") as fps:
                    for c in range(8 * SEGMENTS[-1][1]):
                        emit_outproj(prev[0], prev[1], c, pool=fps,
                                     tag="opf", bufs=3)

    nc.compile()
    return nc


# ---------------------------------------------------------------------------
# Fallback (non-causal mask) — original baseline implementation.
# ---------------------------------------------------------------------------

def _build_fallback():
    NG = ST // 4
    FB_CHUNKS = [(0, 2), (2, 2), (4, 2), (6, 2), (8, 2), (10, 2), (12, 2),
                 (14, 1), (15, 1)]
    nc = bacc.Bacc("TRN2", target_bir_lowering=False, debug=False,
                   enable_asserts=False, num_devices=N_CORES)

    xP = nc.dram_tensor("xP", [128, 2 * DT * (S // 2)], BF, kind="ExternalInput")
    wq = nc.dram_tensor("wq", [HEADS * 128, DT * 128], BF, kind="ExternalInput")
    wk = nc.dram_tensor("wk", [KLOC * 128, DT * 128], BF, kind="ExternalInput")
    wv = nc.dram_tensor("wv", [128, DT * KLOC * H], BF, kind="ExternalInput")
    wo = nc.dram_tensor("wo", [HEADS * H, D], BF, kind="ExternalInput")
    cosT = nc.dram_tensor("cosT", [H, S], FP32, kind="ExternalInput")
    sinST = nc.dram_tensor("sinST", [H, S], FP32, kind="ExternalInput")
    maskf = nc.dram_tensor("maskf", [S, S], FP32, kind="ExternalInput")
    out_sh = nc.dram_tensor("out_shard", [S // TP, D], BF, kind="ExternalOutput")

    with tile.TileContext(nc) as tc:
        with tc.tile_pool(name="persist", bufs=1) as persist, \
             tc.tile_pool(name="dram", bufs=1, space="DRAM") as dram:

            kT_t = [persist.tile([128, S], BF, tag=f"kT{i}", name=f"kT{i}")
                    for i in range(KLOC)]
            v_t = [persist.tile([128, KLOC * H], BF, tag=f"v{i}", name=f"v{i}")
                   for i in range(ST)]
            wo_sb = [persist.tile([128, D], BF, tag=f"wo{i}", name=f"wo{i}")
                     for i in range(HEADS)]
            qT_dram = dram.tile([HEADS * 128, S], BF, tag="qtd", name="qT_dram")
            cc_in = [dram.tile([n * 128, D], BF, tag=f"ccin{g}", name=f"cc_in{g}")
                     for g, (st0, n) in enumerate(FB_CHUNKS)]
            cc_out = [dram.tile([n * 32, D], BF, tag=f"ccout{g}", name=f"cc_out{g}")
                      for g, (st0, n) in enumerate(FB_CHUNKS)]

            with tc.tile_pool(name="p1", bufs=1) as p1, \
                 tc.tile_pool(name="p1ps", bufs=1, space="PSUM") as p1ps:
                ct = p1.tile([H, S], FP32, tag="ct")
                st = p1.tile([H, S], FP32, tag="st")
                wv_sb = p1.tile([128, DT * KLOC * H], BF, tag="wvsb")

                for half in range(2):
                    scols = (half * (S // 2), (half + 1) * (S // 2))
                    xth_t = [p1.tile([128, 8 * (S // 2)], BF, tag="xth", bufs=4,
                                     name=f"xth{half}_{qq}") for qq in range(4)]

                    def xth_dma(qq, split=False):
                        base = (half * DT + qq * 8) * (S // 2)
                        if split:
                            hw_ = 4 * (S // 2)
                            nc.sync.dma_start(xth_t[qq][:, :hw_],
                                              xP.ap()[:, base: base + hw_])
                            nc.sync.dma_start(xth_t[qq][:, hw_:],
                                              xP.ap()[:, base + hw_: base + 8 * (S // 2)])
                        else:
                            nc.sync.dma_start(
                                xth_t[qq][:],
                                xP.ap()[:, base: base + 8 * (S // 2)])

                    if half == 1:
                        for qq in range(4):
                            xth_dma(qq)

                    def xth(d, a, b):
                        return xth_t[d // 8][:, (d % 8) * (S // 2) + a:
                                             (d % 8) * (S // 2) + b]

                    for h in range(HEADS + KLOC):
                        wsrc = wq.ap()[h * 128:(h + 1) * 128, :] if h < HEADS \
                            else wk.ap()[(h - HEADS) * 128:(h - HEADS + 1) * 128, :]
                        if half == 0 and h == 0:
                            xth_dma(0, split=True)
                        wslab = p1.tile([128, DT * 128], BF, tag="wslab", bufs=2)
                        nc.sync.dma_start(wslab[:], wsrc)
                        if half == 0 and h == 0:
                            nc.sync.dma_start(ct[:], cosT.ap())
                            nc.sync.dma_start(st[:], sinST.ap())
                            for qq in range(1, 4):
                                xth_dma(qq)
                            nc.sync.dma_start(wv_sb[:], wv.ap())
                        for sc in range(2):
                            lo = sc * 512
                            qp = p1ps.tile([128, 512], FP32, tag="qp", bufs=3)
                            for d in range(DT):
                                nc.tensor.matmul(
                                    qp[:],
                                    lhsT=wslab[:, d * 128:(d + 1) * 128],
                                    rhs=xth(d, lo, lo + 512),
                                    start=(d == 0), stop=(d == DT - 1))
                            gcol = scols[0] + lo
                            t1 = p1.tile([128, 512], FP32, tag="t1", bufs=2)
                            nc.vector.tensor_mul(t1[:], qp[:], ct[:, gcol:gcol + 512])
                            t2 = p1.tile([128, 512], FP32, tag="t2", bufs=2)
                            nc.vector.tensor_mul(t2[0:64, :], qp[64:128, :],
                                                 st[0:64, gcol:gcol + 512])
                            nc.vector.tensor_mul(t2[64:128, :], qp[0:64, :],
                                                 st[64:128, gcol:gcol + 512])
                            if h < HEADS:
                                robf = p1.tile([128, 512], BF, tag="robf", bufs=2)
                                nc.vector.tensor_add(robf[:], t1[:], t2[:])
                                nc.sync.dma_start(
                                    qT_dram[h * 128:(h + 1) * 128, gcol:gcol + 512],
                                    robf[:])
                            else:
                                nc.vector.tensor_add(
                                    kT_t[h - HEADS][:, gcol:gcol + 512], t1[:], t2[:])

                    for stl in range(ST // 2):
                        sti = half * (ST // 2) + stl
                        vp = p1ps.tile([128, KLOC * H], FP32, tag="vp", bufs=2)
                        for d in range(DT):
                            nc.tensor.matmul(
                                vp[:],
                                lhsT=xth(d, stl * 128, (stl + 1) * 128),
                                rhs=wv_sb[:, d * KLOC * H:(d + 1) * KLOC * H],
                                start=(d == 0), stop=(d == DT - 1))
                        nc.scalar.copy(v_t[sti][:], vp[:])

                for i in range(HEADS):
                    nc.sync.dma_start(wo_sb[i][:], wo.ap()[i * 128:(i + 1) * 128, :])

            with tc.tile_pool(name="p2", bufs=1) as p2, \
                 tc.tile_pool(name="p2ps", bufs=1, space="PSUM") as p2ps:
                ident = p2.tile([128, 128], BF, tag="ident")
                make_identity(nc, ident[:])

                qg_all = [p2.tile([128, S], BF, tag=f"qga{h}", name=f"qga{h}")
                          for h in range(HEADS)]
                for h in range(HEADS):
                    nc.sync.dma_start(qg_all[h][:], qT_dram[h * 128:(h + 1) * 128, :])

                segs = [(0, 4), (4, 4), (8, 4), (12, 4)]
                for t0, nt in segs:
                    W = nt * 128
                    mrow = [p2.tile([128, S], FP32, tag="mrow", bufs=4,
                                    name=f"mrow{t0}_{it}") for it in range(nt)]
                    for it in range(nt):
                        i = t0 + it
                        nc.sync.dma_start(mrow[it][:], maskf.ap()[i * 128:(i + 1) * 128, :])

                    yT_sb = [p2.tile([128, W], BF, tag=f"yt{h}", bufs=2,
                                     name=f"yt{t0}_{h}") for h in range(HEADS)]
                    for h in range(HEADS):
                        kv = h % KLOC
                        nquad = NG
                        pTq = [p2.tile([128, 4 * W], BF, tag=f"ptq{q}", bufs=1,
                                       name=f"ptq{t0}_{h}_{q}") for q in range(nquad)]
                        for it in range(nt):
                            i = t0 + it
                            nsk = S
                            prow = p2.tile([128, S], BF, tag="prow", bufs=3)
                            sums = []
                            nch = (nsk + 1023) // 1024
                            for c in range(nch):
                                w = min(1024, nsk - c * 1024)
                                sp = p2ps.tile([128, 1024], FP32, tag="sp", bufs=2)
                                for cc in range((w + 511) // 512):
                                    ww = min(512, w - cc * 512)
                                    o = cc * 512
                                    nc.tensor.matmul(
                                        sp[:, o:o + ww],
                                        lhsT=qg_all[h][:, i * 128:(i + 1) * 128],
                                        rhs=kT_t[kv][:, c * 1024 + o: c * 1024 + o + ww],
                                        start=True, stop=True)
                                nc.vector.tensor_add(
                                    sp[:, :w], sp[:, :w],
                                    mrow[it][:, c * 1024: c * 1024 + w])
                                sm = p2.tile([128, 1], FP32, tag="sm", bufs=8)
                                nc.scalar.activation(
                                    prow[:, c * 1024: c * 1024 + w], sp[:, :w],
                                    mybir.ActivationFunctionType.Exp, accum_out=sm[:])
                                sums.append(sm)
                            if nch == 2:
                                tot = p2.tile([128, 1], FP32, tag="tot", bufs=4)
                                nc.vector.tensor_add(tot[:], sums[0][:], sums[1][:])
                            else:
                                tot = sums[0]
                            rc = p2.tile([128, 1], FP32, tag="rc", bufs=4)
                            nc.vector.reciprocal(rc[:], tot[:])
                            diag = p2.tile([128, 128], BF, tag="diag", bufs=4)
                            nc.vector.tensor_scalar_mul(diag[:], ident[:], rc[:])
                            jtop = ST - 1
                            for qd in range(jtop // 4 + 1):
                                jlo, jhi = 4 * qd, min(4 * qd + 3, jtop)
                                nq = jhi - jlo + 1
                                tpp = p2ps.tile([128, 512], FP32, tag="tp", bufs=2)
                                for j in range(jlo, jhi + 1):
                                    nc.tensor.matmul(
                                        tpp[:, (j - jlo) * 128:(j - jlo + 1) * 128],
                                        lhsT=prow[:, j * 128:(j + 1) * 128],
                                        rhs=diag[:], start=True, stop=True)
                                pt_dst = pTq[qd][:].rearrange("p (a b) -> p a b", a=4)[
                                    :, 0:nq, it * 128:(it + 1) * 128]
                                pt_src = tpp[:, :nq * 128].rearrange(
                                    "p (a b) -> p a b", b=128)
                                if (it + qd) % 2:
                                    nc.scalar.copy(pt_dst, pt_src)
                                else:
                                    nc.vector.tensor_copy(pt_dst, pt_src)
                        yp = p2ps.tile([128, W], FP32, tag="yp", bufs=1)
                        jmax = ST
                        for j in range(jmax):
                            lo = 0
                            nc.tensor.matmul(
                                yp[:, lo:W],
                                lhsT=v_t[j][:, kv * H:(kv + 1) * H],
                                rhs=pTq[j // 4][:, (j % 4) * W + lo: (j % 4) * W + W],
                                start=(j == 0), stop=(j == jmax - 1))
                        nc.scalar.copy(yT_sb[h][:], yp[:])

                    for it in range(nt):
                        i = t0 + it
                        cg = next(ci for ci, (cs, cn) in enumerate(FB_CHUNKS)
                                  if cs <= i < cs + cn)
                        c_start, c_n = FB_CHUNKS[cg]
                        for dc in range(8):
                            op = p2ps.tile([128, 512], FP32, tag="op", bufs=1)
                            for hh in range(HEADS):
                                nc.tensor.matmul(
                                    op[:],
                                    lhsT=yT_sb[hh][:, it * 128:(it + 1) * 128],
                                    rhs=wo_sb[hh][:, dc * 512:(dc + 1) * 512],
                                    start=(hh == 0), stop=(hh == HEADS - 1))
                            oev = p2.tile([128, 512], BF, tag="oev", bufs=16)
                            nc.scalar.copy(oev[:], op[:])
                            nc.sync.dma_start(
                                cc_in[cg][(i - c_start) * 128:(i - c_start + 1) * 128,
                                          dc * 512:(dc + 1) * 512],
                                oev[:])
                        if i == c_start + c_n - 1:
                            nc.gpsimd.collective_compute(
                                "ReduceScatter", mybir.AluOpType.add,
                                replica_groups=[[0, 1, 2, 3], [4, 5, 6, 7]],
                                ins=[cc_in[cg].opt()], outs=[cc_out[cg].opt()])
                            orow = sum(cn * 32 for cs, cn in FB_CHUNKS[:cg])
                            nc.sync.dma_start(
                                out_sh.ap()[orow: orow + c_n * 32, :], cc_out[cg][:])

    nc.compile()
    return nc


_CANON_MASK = None


def _is_causal(mask: np.ndarray) -> bool:
    global _CANON_MASK
    if _CANON_MASK is None:
        _CANON_MASK = np.triu(np.full((S, S), -1e9, dtype=np.float32), k=1)
    return mask.shape == (S, S) and np.array_equal(mask, _CANON_MASK)


def _prepare(x, wq, wk, wv, wo, mask, sin, cos):
    causal = _is_causal(np.asarray(mask, dtype=np.float32))
    if causal not in _CACHE:
        _CACHE[causal] = _build_fast() if causal else _build_fallback()
    nc = _CACHE[causal]

    x = np.asarray(x, dtype=np.float32)
    scale = np.float32(H ** -0.5)
    cosT = np.ascontiguousarray(np.asarray(cos, np.float32).T)          # [H, S]
    sinT = np.asarray(sin, np.float32).T.copy()                          # [H, S]
    sinT[0:H // 2] = -sinT[0:H // 2]                                     # signed
    # per-core weight shards; head order = r-major over local kv heads
    in_maps = []
    for c in range(N_CORES):
        b, tp = c // TP, c % TP
        ks = slice(tp * KLOC, (tp + 1) * KLOC)
        wq_c = np.asarray(wq, np.float32)[:, :, ks, :].reshape(D, HEADS * H)
        wk_c = (np.asarray(wk, np.float32)[:, ks, :] * scale).reshape(D, KLOC * H)
        wv_c = np.asarray(wv, np.float32)[:, ks, :].reshape(D, KLOC * H)
        if causal:
            xp = x[b].reshape(NQ, 512, DT, 128).transpose(3, 0, 2, 1) \
                     .reshape(128, NQ * DT * 512).astype(BF16)
        else:
            xp = x[b].reshape(2, S // 2, DT, 128).transpose(3, 0, 2, 1) \
                     .reshape(128, 2 * DT * (S // 2)).astype(BF16)
        m = {
            "xP": xp,
            "wq": wq_c.reshape(DT, 128, HEADS, H).transpose(2, 1, 0, 3)
                      .reshape(HEADS * 128, DT * 128).astype(BF16),
            "wk": wk_c.reshape(DT, 128, KLOC, H).transpose(2, 1, 0, 3)
                      .reshape(KLOC * 128, DT * 128).astype(BF16),
            "wv": wv_c.reshape(DT, 128, KLOC * H).transpose(1, 0, 2)
                      .reshape(128, DT * KLOC * H).astype(BF16),
            "wo": np.asarray(wo, np.float32)[:, ks, :, :].reshape(HEADS * H, D).astype(BF16),
            "cosT": cosT,
            "sinST": sinT,
        }
        if causal:
            md = np.empty((128, S), np.float32)
            for i in range(ST):
                md[:, i * 128:(i + 1) * 128] = mask[i * 128:(i + 1) * 128,
                                                    i * 128:(i + 1) * 128].T
            m["maskdT"] = md.astype(BF16)
        else:
            m["maskf"] = np.asarray(mask, np.float32)
        in_maps.append(m)
    return nc, in_maps, causal


def _assemble(results, causal):
    chunks = CC_CHUNKS if causal else \
        [(0, 2), (2, 2), (4, 2), (6, 2), (8, 2), (10, 2), (12, 2), (14, 1), (15, 1)]
    out = np.empty((B, S, D), dtype=np.float32)
    for c in range(N_CORES):
        b, tp = c // TP, c % TP
        sh = results[c]["out_shard"].astype(np.float32)
        if causal:
            for cs, cn in chunks:
                rows = cn * 32
                out[b, cs * 128 + tp * rows: cs * 128 + (tp + 1) * rows, :] = \
                    sh[cs * 32: cs * 32 + rows]
        else:
            orow = 0
            for cs, cn in chunks:
                rows = cn * 32
                out[b, cs * 128 + tp * rows: cs * 128 + (tp + 1) * rows, :] = \
                    sh[orow: orow + rows]
                orow += rows
    return out


def kernel(x, wq, wk, wv, wo, mask, sin, cos):
    nc, in_maps, causal = _prepare(x, wq, wk, wv, wo, mask, sin, cos)
    try:
        res = bass_utils.run_bass_kernel_spmd(nc, in_maps,
                                              core_ids=list(range(N_CORES)))
    except Exception:
        # transient device-side failures (e.g. NRT exec-unit errors) have
        # been observed once; a clean re-run succeeds.
        import time as _time
        _time.sleep(2.0)
        res = bass_utils.run_bass_kernel_spmd(nc, in_maps,
                                              core_ids=list(range(N_CORES)))
    return _assemble(res.results, causal)


def _traced_run(x, wq, wk, wv, wo, mask, sin, cos):
    """Like kernel() but with NTFF tracing; returns BassKernelResults."""
    nc, in_maps, causal = _prepare(x, wq, wk, wv, wo, mask, sin, cos)
    res = bass_utils.run_bass_kernel_spmd(nc, in_maps, core_ids=list(range(N_CORES)),
                                          trace=True)
    res.full_output = _assemble(res.results, causal)
    return res
